# revision 15
# baseline (speedup 1.0000x reference)
# Multi-head self-attention kernel for Trainium2, 8 NeuronCores.
# Sharding: data-parallel over batch (b=8 -> one batch per core).
#
# v3: software-pipelined single pass, fp16 inputs, streamed startup.
#  - Inputs cast to fp16 on host; hsT split across both HWDGE rings in
#    chunk-granular DMAs so the first projection streams behind the DMA.
#  - First Q/K projection consumes hsT chunks in arrival order.
#  - Scores: the two head-halves (PE row groups 0:64 / 64:128) are emitted
#    adjacently so their matmuls run concurrently on the array.
#  - exp() runs as 64x [128, 2048] ACTIVATEs on a merged [P, 2, 1024] score
#    PSUM tile (fewer ACT instructions -> less fixed overhead).
#  - V projection split into e_out halves, spread across early stages;
#    PV(p) (heads 2p, 2p+1) only needs its own e_out quarter.
#  - PV: exp chunk stationary (FWL), moving [v_h | 1] (65 cols); softmax
#    denominator lands in PSUM col 64; DVE fuses divide+bias via
#    scalar_tensor_tensor.
# Output DRAM [H, L, D] fp16 per core == reference's out.reshape(L, H*D).

import numpy as np

B, L, E = 8, 1024, 1024
H, D = 16, 64
NC = 8          # cores
P = 128         # partitions
CH = E // P     # 8 contraction chunks
MT = E // P     # 8 output tiles (e_out) == head pairs
LT = L // P     # 8 l-tiles
HPT = P // D    # 2 heads per 128-partition tile
VW = D + 2      # 66: v cols per head (64 + ones col + pad)

TRACE = False
_cached = {}


def _build():
    import concourse.bacc as bacc
    import concourse.mybir as mybir
    import concourse.tile as tile
    import concourse.bass as bass

    F32 = mybir.dt.float32
    F16 = mybir.dt.float16
    Exp = mybir.ActivationFunctionType.Exp
    Mult = mybir.AluOpType.mult
    Add = mybir.AluOpType.add

    nc = bacc.Bacc("TRN2", target_bir_lowering=False, debug=False)
    hsT = nc.dram_tensor("hsT", [E, L], F16, kind="ExternalInput").ap()
    wqT = nc.dram_tensor("wqT", [E, E], F16, kind="ExternalInput").ap()
    wkT = nc.dram_tensor("wkT", [E, E], F16, kind="ExternalInput").ap()
    wvT = nc.dram_tensor("wvT", [E, E], F16, kind="ExternalInput").ap()
    bq = nc.dram_tensor("bq", [E], F32, kind="ExternalInput").ap()
    bk = nc.dram_tensor("bk", [E], F32, kind="ExternalInput").ap()
    bv = nc.dram_tensor("bv", [E], F32, kind="ExternalInput").ap()
    out = nc.dram_tensor("out", [H, L, D], F16, kind="ExternalOutput").ap()

    with tile.TileContext(nc) as tc:
        with tc.tile_pool(name="big", bufs=1) as big, \
             tc.tile_pool(name="wpool", bufs=4) as wpool, \
             tc.tile_pool(name="epool", bufs=3) as epool, \
             tc.tile_pool(name="spool", bufs=3) as spool, \
             tc.tile_pool(name="pjp", bufs=1, space="PSUM") as pjp, \
             tc.tile_pool(name="scp", bufs=3, space="PSUM") as scp, \
             tc.tile_pool(name="pvp", bufs=1, space="PSUM") as pvp:

            wts = {}

            def load_w(m, which, eng):
                wT = wqT if which == 0 else wkT
                t = wpool.tile([P, CH, P], F16, tag="w", name=f"w{which}_{m}")
                eng.dma_start(
                    out=t,
                    in_=wT[:, m * P:(m + 1) * P].rearrange(
                        "(c p) n -> p c n", p=P))
                wts[(m, which)] = t

            # ---- input DMAs ----
            # sync ring: w00, w01, bq, bk, hsT chunks 0-3, w10, w11
            # scalar ring: hsT chunks 4-7, wv, bv_bc (ACT idle this early)
            # ring transfers are FIFO with ~2us fixed cost per DMA, so order
            # strictly by first use: w00 | hsT lower half, then the rest.
            hsT_sb = big.tile([P, CH, L], F16)
            load_w(0, 0, nc.sync)
            nc.sync.dma_start(
                out=hsT_sb[:, 0:4, :],
                in_=hsT[:4 * P, :].rearrange("(c p) l -> p c l", p=P))
            nc.scalar.dma_start(
                out=hsT_sb[:, 4:CH, :],
                in_=hsT[4 * P:, :].rearrange("(c p) l -> p c l", p=P))
            load_w(0, 1, nc.scalar)
            bq_sb = big.tile([P, MT], F32)
            bk_sb = big.tile([P, MT], F32)
            nc.sync.dma_start(out=bq_sb, in_=bq.rearrange("(m p) -> p m", p=P))
            nc.sync.dma_start(out=bk_sb, in_=bk.rearrange("(m p) -> p m", p=P))
            load_w(1, 0, nc.sync)
            load_w(1, 1, nc.sync)

            wv_sb = big.tile([P, CH, E], F16)
            nc.scalar.dma_start(
                out=wv_sb, in_=wvT.rearrange("(c p) l -> p c l", p=P))
            bv_bc = big.tile([P, E], F32)
            nc.scalar.dma_start(
                out=bv_bc,
                in_=bass.AP(tensor=bv.tensor, offset=0, ap=[[0, P], [1, E]]))

            # ---- resident SBUF tensors ----
            qT_sb = big.tile([P, MT, L], F16)       # [p(e_out in tile), m, lq]
            kT_sb = big.tile([P, MT, L], F16)
            v_sb = big.tile([P, LT, H * VW], F16)   # [p(l in tile), m, h*66+c]
            v4 = v_sb.rearrange("p m (h c) -> p m h c", h=H)
            nc.vector.memset(v4[:, :, :, D:VW], 1.0)

            # warm the ACT exp table (~2.7us ACT_TABLE_LOAD) before scores
            warm = spool.tile([P, 1], F16, tag="warm", name="warm")
            nc.scalar.activation(warm, v4[:, 0, 0, D:D + 1], Exp, scale=1.0)

            CO = [4, 5, 6, 7, 0, 1, 2, 3]   # hsT chunk arrival order

            def emit_qk(m, chunk_order=None):
                co = chunk_order or list(range(CH))
                for which, (dst, bias) in enumerate(
                        ((qT_sb, bq_sb), (kT_sb, bk_sb))):
                    wt = wts.pop((m, which))
                    if chunk_order:
                        # stream both n-halves per chunk as chunks arrive
                        pss = [pjp.tile([P, 512], F32, tag="pj",
                                        name=f"psqk{m}_{which}_{n}")
                               for n in range(2)]
                        for i, c in enumerate(co):
                            for n in range(2):
                                nc.tensor.matmul(
                                    pss[n], wt[:, c, :],
                                    hsT_sb[:, c, n * 512:(n + 1) * 512],
                                    start=(i == 0), stop=(i == CH - 1))
                        for n in range(2):
                            nc.vector.tensor_scalar_add(
                                dst[:, m, n * 512:(n + 1) * 512], pss[n],
                                bias[:, m:m + 1])
                    else:
                        for n in range(2):
                            ps = pjp.tile([P, 512], F32, tag="pj",
                                          name=f"psqk{m}_{which}_{n}")
                            for c in range(CH):
                                nc.tensor.matmul(
                                    ps, wt[:, c, :],
                                    hsT_sb[:, c, n * 512:(n + 1) * 512],
                                    start=(c == 0), stop=(c == CH - 1))
                            nc.vector.tensor_scalar_add(
                                dst[:, m, n * 512:(n + 1) * 512], ps,
                                bias[:, m:m + 1])

            def emit_v(m, n):
                # l-tile m, e_out half n (heads 8n .. 8n+7)
                ps = pjp.tile([P, 512], F32, tag="pj", name=f"psv{m}_{n}")
                for c in range(CH):
                    nc.tensor.matmul(
                        ps, hsT_sb[:, c, m * P:(m + 1) * P],
                        wv_sb[:, c, n * 512:(n + 1) * 512],
                        start=(c == 0), stop=(c == CH - 1))
                nc.vector.tensor_copy(
                    v4[:, m, n * 8:(n + 1) * 8, 0:D],
                    ps.rearrange("p (h c) -> p h c", h=8))

            def emit_sc(p_i):
                # scores + exp for head pair p_i; halves emitted adjacently
                # (disjoint PE row groups -> concurrent matmuls)
                e = epool.tile([P, CH, HPT, L], F16, tag="e", name=f"e{p_i}")
                for c in range(CH):
                    scs = [scp.tile([P, L], F32, tag="sc",
                                    name=f"sc{p_i}_{c}_{half}")
                           for half in range(HPT)]
                    for n in range(2):
                        for half in range(HPT):
                            lo = half * D
                            nc.tensor.matmul(
                                scs[half][:, n * 512:(n + 1) * 512],
                                kT_sb[lo:lo + D, p_i, c * P:(c + 1) * P],
                                qT_sb[lo:lo + D, p_i, n * 512:(n + 1) * 512],
                                start=True, stop=True)
                    for half in range(HPT):
                        nc.scalar.activation(
                            e[:, c, half], scs[half], Exp, scale=0.125)
                return e

            def emit_pv(p_i, e, st):
                for t in range(LT):
                    pv = pvp.tile([P, HPT, 68], F32, tag="pv",
                                  name=f"pv{p_i}_{t}")
                    for half in range(HPT):
                        h = 2 * p_i + half
                        for c in range(CH):
                            nc.tensor.matmul(
                                pv[:, half, 0:D + 1],
                                e[:, c, half, t * P:(t + 1) * P],
                                v_sb[:, c, h * VW:h * VW + D + 1],
                                start=(c == 0), stop=(c == CH - 1))
                    for half in range(HPT):
                        h = 2 * p_i + half
                        rs = spool.tile([P, 1], F32, tag="rs",
                                        name=f"rs{p_i}_{t}_{half}")
                        nc.vector.reciprocal(rs, pv[:, half, D:D + 1])
                        nc.vector.scalar_tensor_tensor(
                            st[:, half, t, :], pv[:, half, 0:D], rs,
                            bv_bc[:, h * D:(h + 1) * D], Mult, Add)

            def emit_out(p_i, st):
                for half in range(HPT):
                    h = 2 * p_i + half
                    nc.sync.dma_start(
                        out=out[h].rearrange("(t p) d -> p t d", p=P),
                        in_=st[:, half])

            # ---- pipelined emission ----
            emit_qk(0, chunk_order=CO)

            # V half-schedule per stage: (m, n) pairs
            vsched = {
                0: [(0, 0), (1, 0), (2, 0), (3, 0)],
                1: [(4, 0), (5, 0), (6, 0), (7, 0)],
                2: [(0, 1), (1, 1)],
                3: [(2, 1), (3, 1)],
                4: [(4, 1), (5, 1)],
                5: [(6, 1), (7, 1)],
            }
            exps = {}
            sts = {}
            for p_i in range(MT):
                exps[p_i] = emit_sc(p_i)
                if p_i + 1 < MT:
                    if p_i + 2 < MT:
                        load_w(p_i + 2, 0, nc.sync)
                        load_w(p_i + 2, 1, nc.sync)
                    emit_qk(p_i + 1)
                for (m, n) in vsched.get(p_i, []):
                    emit_v(m, n)
                pvs = [p_i - 2] if p_i < MT - 1 else [MT - 3, MT - 2]
                for j in pvs:
                    if j < 0:
                        continue
                    sts[j] = spool.tile([P, HPT, LT, D], F16, tag="st",
                                        name=f"st{j}")
                    emit_pv(j, exps.pop(j), sts[j])
                    emit_out(j, sts[j])
            j = MT - 1
            sts[j] = spool.tile([P, HPT, LT, D], F16, tag="st",
                                name=f"st{j}")
            emit_pv(j, exps.pop(j), sts[j])
            emit_out(j, sts[j])

    nc.compile()
    return nc


def _get_nc():
    if "nc" not in _cached:
        _cached["nc"] = _build()
    return _cached["nc"]


def kernel(hidden_states, w_q, b_q, w_k, b_k, w_v, b_v):
    from concourse import bass_utils

    hs = np.asarray(hidden_states, dtype=np.float32)
    b_q = np.asarray(b_q, dtype=np.float32)
    b_k = np.asarray(b_k, dtype=np.float32)
    b_v = np.asarray(b_v, dtype=np.float32)

    nc = _get_nc()
    hsT = np.ascontiguousarray(
        hs.transpose(0, 2, 1)).astype(np.float16)
    wqT = np.ascontiguousarray(np.asarray(w_q, np.float32).T).astype(np.float16)
    wkT = np.ascontiguousarray(np.asarray(w_k, np.float32).T).astype(np.float16)
    wvT = np.ascontiguousarray(np.asarray(w_v, np.float32).T).astype(np.float16)
    in_maps = [
        {"hsT": hsT[i], "wqT": wqT, "wkT": wkT, "wvT": wvT,
         "bq": b_q, "bk": b_k, "bv": b_v}
        for i in range(NC)
    ]
    res = bass_utils.run_bass_kernel_spmd(
        nc, in_maps, core_ids=list(range(NC)), trace=TRACE)
    kernel.last_exec_time_ns = res.exec_time_ns
    kernel.last_results = res.results
    return np.stack(
        [res.results[i]["out"].reshape(L, H * D).astype(np.float32)
         for i in range(NC)])


kernel.last_exec_time_ns = None
